# revision 6
# baseline (speedup 1.0000x reference)
"""Two-layer DGL-style GCN (norm='both') on 8 TRN2 NeuronCores.

Sharding: dst-node blocks of 12544 per core (98 tiles of 128 nodes).

Layer 1 aggregates in INPUT space (agg(xs) @ W1 == agg(xs @ W1)), so no
projection pass over all nodes is needed: per dst tile, gather the
per-edge src rows of xs straight from DRAM with batched SWDGE dma_gather
(a handful of instructions per tile group instead of one indirect DMA
per 128 edges), segment-sum via one-hot matmuls into PSUM, then apply
W1 -> relu -> W2 per tile. Layer-2 projections are exchanged with an
AllGather; layer 2 repeats the gather + one-hot aggregation.

Gather indices are int16, so edges are bucketed by 32768-row src chunk;
each (tile-group, chunk) is one dma_gather instruction.

kernel(**inputs) takes the full unsharded inputs and returns the full
output; all sharding happens inside.
"""

import math

import numpy as np

import concourse.bacc as bacc
import concourse.bass as bass
import concourse.bass_utils as bass_utils
import concourse.mybir as mybir
import concourse.tile as tile
from concourse import library_config

P = 128
CHUNK = 32768  # int16 index reach per dma_gather

N_CORES = 8

F16 = mybir.dt.float16
F32 = mybir.dt.float32
I16 = mybir.dt.int16

# set by test.py to request a profiled run
TRACE = False
LAST_RESULTS = None


# ---------------------------------------------------------------- host prep


def _group_size(ntiles_pc):
    for g in (7, 8, 6, 5, 4, 3, 2, 1):
        if ntiles_pc % g == 0:
            return g
    return 1


def prep_inputs(x, edge_index, W1, W2, ncores):
    """Shard the full inputs -> (in_maps, meta)."""
    n, cin = x.shape
    chid = W1.shape[1]
    cout = W2.shape[1]
    e = edge_index.shape[1]
    assert cin == P and chid == P and cout <= P

    ntiles_pc = math.ceil(n / (ncores * P))
    nb = ntiles_pc * P
    npad = nb * ncores
    ntt = ntiles_pc * ncores
    G = _group_size(ntiles_pc)
    ngrp = ntiles_pc // G

    src = np.asarray(edge_index[0], dtype=np.int64)
    dst = np.asarray(edge_index[1], dtype=np.int64)

    deg_out = np.bincount(src, minlength=npad).astype(np.float32)
    deg_in = np.bincount(dst, minlength=npad).astype(np.float32)
    oi = 1.0 / np.sqrt(np.maximum(deg_out, 1.0))
    ii = 1.0 / np.sqrt(np.maximum(deg_in, 1.0))

    # out-degree scale folded into x; row-major fp16 (gather source rows)
    xsr = np.zeros((npad, P), dtype=np.float16)
    xsr[:n] = np.asarray(x, dtype=np.float32) * oi[:n, None]

    # src chunks of <=32768 rows (int16 gather index reach)
    nch = math.ceil(npad / CHUNK)
    CH = [min(k * CHUNK, npad) for k in range(nch + 1)]

    # sort edges by (dst tile, src chunk, src)
    tg = dst // P
    kk = np.minimum(src // CHUNK, nch - 1)
    order = np.lexsort((src, kk, tg))
    tg_s = tg[order]
    kk_s = kk[order]
    src_s = src[order]
    dloc_s = (dst[order] % P).astype(np.float16)

    gkey = tg_s * nch + kk_s
    counts = np.bincount(gkey, minlength=ntt * nch)
    starts = np.zeros(ntt * nch + 1, dtype=np.int64)
    starts[1:] = np.cumsum(counts)
    pos = np.arange(e, dtype=np.int64) - starts[gkey]

    cnt2 = counts.reshape(ntt, nch)
    B = [int(math.ceil(cnt2[:, k].max() / P)) for k in range(nch)]
    PRE = np.concatenate([[0], np.cumsum(B)[:-1]]).astype(np.int64)
    NBLK = int(sum(B))

    Bv = np.array(B, dtype=np.int64)
    PREv = PRE

    # flat slot id in (group, chunk, tile_local, block, p) instruction order
    core_s = tg_s // ntiles_pc
    tloc_s = tg_s - core_s * ntiles_pc
    g_s = tloc_s // G
    tl_s = tloc_s % G
    grp_slots = G * NBLK * P
    F = g_s * grp_slots + G * PREv[kk_s] * P + tl_s * Bv[kk_s] * P + pos

    totslot = ntiles_pc * NBLK * P  # per core
    totcol = totslot // 16

    idx_flat = np.zeros((ncores, totslot), dtype=np.int16)
    idx_flat[core_s, F] = (src_s - np.array(CH, dtype=np.int64)[kk_s]).astype(np.int16)

    # dstloc: [128, ntiles_pc*NBLK], col = t*NBLK + PRE[k] + b
    dl_flat = np.full((ncores, ntiles_pc * NBLK * P), -1.0, dtype=np.float16)
    col_e = tloc_s * NBLK + PREv[kk_s] + pos // P
    dl_flat[core_s, col_e * P + (pos % P)] = dloc_s

    # each of the 8 gpsimd cores reads its own 16-partition stripe: replicate
    idx_pc = np.tile(
        idx_flat.reshape(ncores, totcol, 16).transpose(0, 2, 1), (1, 8, 1)
    ).copy()
    dl_pc = (
        dl_flat.reshape(ncores, ntiles_pc * NBLK, P).transpose(0, 2, 1).copy()
    )

    iio = (ii * oi).reshape(ncores, ntiles_pc, P).transpose(0, 2, 1).copy()
    ii2 = ii.reshape(ncores, ntiles_pc, P).transpose(0, 2, 1).copy()

    W1_16 = np.asarray(W1, dtype=np.float16)
    W2p = np.zeros((chid, P), dtype=np.float16)
    W2p[:, :cout] = np.asarray(W2, dtype=np.float16)

    Bmax = max(B)
    iota_rep = np.tile(np.arange(P, dtype=np.float16), (P, Bmax))

    in_maps = [
        {
            "xsr": xsr,
            "idx": idx_pc[c],
            "dl": dl_pc[c],
            "iio": iio[c].astype(np.float32),
            "ii2": ii2[c].astype(np.float32),
            "W1": W1_16,
            "W2": W2p,
            "iota_rep": iota_rep,
        }
        for c in range(ncores)
    ]

    meta = dict(
        n=n, cin=cin, chid=chid, cout=cout,
        ncores=ncores, ntiles_pc=ntiles_pc, nb=nb, npad=npad,
        nch=nch, CH=tuple(CH), B=tuple(B), NBLK=NBLK, G=G, ngrp=ngrp,
        totcol=totcol,
    )
    return in_maps, meta


# ---------------------------------------------------------------- device program


def build_nc(meta, debug=False, enable_asserts=False):
    ncores = meta["ncores"]
    ntiles_pc = meta["ntiles_pc"]
    nb = meta["nb"]
    npad = meta["npad"]
    nch = meta["nch"]
    CH = meta["CH"]
    B = meta["B"]
    NBLK = meta["NBLK"]
    G = meta["G"]
    ngrp = meta["ngrp"]
    totcol = meta["totcol"]
    PRE = np.concatenate([[0], np.cumsum(B)[:-1]]).astype(np.int64)
    Bmax = max(B)

    nc = bacc.Bacc(
        "TRN2",
        target_bir_lowering=False,
        debug=debug,
        enable_asserts=enable_asserts,
        num_devices=ncores,
    )

    xsr = nc.dram_tensor("xsr", [npad, P], F16, kind="ExternalInput")
    idx_d = nc.dram_tensor("idx", [P, totcol], I16, kind="ExternalInput")
    dl_d = nc.dram_tensor("dl", [P, ntiles_pc * NBLK], F16, kind="ExternalInput")
    iio_d = nc.dram_tensor("iio", [P, ntiles_pc], F32, kind="ExternalInput")
    ii2_d = nc.dram_tensor("ii2", [P, ntiles_pc], F32, kind="ExternalInput")
    W1_d = nc.dram_tensor("W1", [P, P], F16, kind="ExternalInput")
    W2_d = nc.dram_tensor("W2", [P, P], F16, kind="ExternalInput")
    iota_d = nc.dram_tensor("iota_rep", [P, Bmax * P], F16, kind="ExternalInput")

    out = nc.dram_tensor("out", [nb, 64], F32, kind="ExternalOutput")

    H2b = nc.dram_tensor("H2b", [nb, P], F16)
    H2f = nc.dram_tensor("H2f", [npad, P], F16, addr_space="Shared")

    grp_cols = G * NBLK  # G-tile blocks per group

    with tile.TileContext(nc) as tc:
        with (
            tc.tile_pool(name="const", bufs=1) as cpool,
            tc.tile_pool(name="gbuf", bufs=2) as gpool,
            tc.tile_pool(name="mbuf", bufs=8) as mpool,
            tc.tile_pool(name="agg_ps", bufs=2, space="PSUM") as pspool,
            tc.tile_pool(name="aux_ps", bufs=2, space="PSUM") as xpspool,
            tc.tile_pool(name="flush", bufs=3) as flpool,
        ):
            nc.gpsimd.load_library(library_config.mlp)

            w1_sb = cpool.tile([P, P], F16)
            nc.sync.dma_start(w1_sb[:], W1_d.ap())
            w2_sb = cpool.tile([P, P], F16)
            nc.sync.dma_start(w2_sb[:], W2_d.ap())
            iota_sb = cpool.tile([P, Bmax * P], F16)
            nc.sync.dma_start(iota_sb[:], iota_d.ap())
            idx_sb = cpool.tile([P, totcol], I16)
            nc.sync.dma_start(idx_sb[:], idx_d.ap())
            dl_sb = cpool.tile([P, ntiles_pc * NBLK], F16)
            nc.sync.dma_start(dl_sb[:], dl_d.ap())
            iio_sb = cpool.tile([P, ntiles_pc], F32)
            nc.sync.dma_start(iio_sb[:], iio_d.ap())
            ii2_sb = cpool.tile([P, ntiles_pc], F32)
            nc.sync.dma_start(ii2_sb[:], ii2_d.ap())

            def gathers(gtile, src_dram, g):
                gv = gtile[:].rearrange("p (b e) -> p b e", e=P)
                for k in range(nch):
                    if B[k] == 0:
                        continue
                    nidx = G * B[k] * P
                    c0 = g * (grp_cols * P // 16) + int(G * PRE[k] * P) // 16
                    nc.gpsimd.dma_gather(
                        gv[:, G * int(PRE[k]) : G * int(PRE[k] + B[k]), :],
                        src_dram.ap()[CH[k] : CH[k + 1], :],
                        idx_sb[:, c0 : c0 + nidx // 16],
                        nidx,
                        nidx,
                        P,
                        single_packet=False,
                    )

            def build_m(t, k):
                M = mpool.tile([P, B[k] * P], F16, tag="m")
                c0 = t * NBLK + int(PRE[k])
                in0 = iota_sb[:, : B[k] * P].rearrange("p (b e) -> p b e", e=P)
                in1 = dl_sb[:, c0 : c0 + B[k]].rearrange("p (b e) -> p b e", e=1)
                in0b, in1b = bass.broadcast_tensor_aps(in0, in1)
                nc.vector.tensor_tensor(
                    out=M[:].rearrange("p (b e) -> p b e", e=P),
                    in0=in0b,
                    in1=in1b,
                    op=mybir.AluOpType.is_equal,
                )
                return M

            # ---- Layer 1: aggregate xs per dst tile, project W1->relu->W2
            for g in range(ngrp):
                G1 = gpool.tile([P, grp_cols * P], F16, tag="g")
                gathers(G1, xsr, g)
                g1v = G1[:].rearrange("p (b e) -> p b e", e=P)
                for tl in range(G):
                    t = g * G + tl
                    psT = pspool.tile([P, P], F32, tag="psT")
                    mm = 0
                    for k in range(nch):
                        if B[k] == 0:
                            continue
                        M = build_m(t, k)
                        for b in range(B[k]):
                            bc = G * int(PRE[k]) + tl * B[k] + b
                            nc.tensor.matmul(
                                psT[:],
                                lhsT=g1v[:, bc, :],
                                rhs=M[:, b * P : (b + 1) * P],
                                start=(mm == 0),
                                stop=(mm == NBLK - 1),
                            )
                            mm += 1
                    # psT = agg_x^T [cin, d]
                    aggT = flpool.tile([P, P], F16, tag="aggT")
                    nc.vector.tensor_copy(aggT[:], psT[:])
                    h1pT = xpspool.tile([P, P], F32, tag="h1pT")
                    nc.tensor.matmul(
                        h1pT[:], lhsT=w1_sb[:], rhs=aggT[:], start=True, stop=True
                    )
                    # x2^T = relu(h1pT) [chid, d] (iio scale commutes to the end)
                    x2T = flpool.tile([P, P], F16, tag="x2T")
                    nc.scalar.activation(
                        x2T[:], h1pT[:], mybir.ActivationFunctionType.Relu
                    )
                    h2p = xpspool.tile([P, P], F32, tag="h2p")
                    nc.tensor.matmul(
                        h2p[:], lhsT=x2T[:], rhs=w2_sb[:], start=True, stop=True
                    )
                    h2s = flpool.tile([P, P], F16, tag="h2s")
                    nc.vector.tensor_scalar(
                        out=h2s[:],
                        in0=h2p[:],
                        scalar1=iio_sb[:, t : t + 1],
                        scalar2=None,
                        op0=mybir.AluOpType.mult,
                    )
                    nc.sync.dma_start(H2b.ap()[t * P : (t + 1) * P, :], h2s[:])

            # ---- exchange layer-2 projections
            nc.gpsimd.collective_compute(
                "AllGather",
                mybir.AluOpType.bypass,
                replica_groups=[list(range(ncores))],
                ins=[H2b.ap().opt()],
                outs=[H2f.ap().opt()],
            )

            # ---- Layer 2: aggregate H2f per dst tile
            for g in range(ngrp):
                G2 = gpool.tile([P, grp_cols * P], F16, tag="g")
                gathers(G2, H2f, g)
                g2v = G2[:].rearrange("p (b e) -> p b e", e=P)
                for tl in range(G):
                    t = g * G + tl
                    ps2 = pspool.tile([P, 64], F32, tag="ps2")
                    mm = 0
                    for k in range(nch):
                        if B[k] == 0:
                            continue
                        M = build_m(t, k)
                        for b in range(B[k]):
                            bc = G * int(PRE[k]) + tl * B[k] + b
                            nc.tensor.matmul(
                                ps2[:],
                                lhsT=M[:, b * P : (b + 1) * P],
                                rhs=g2v[:, bc, 0:64],
                                start=(mm == 0),
                                stop=(mm == NBLK - 1),
                            )
                            mm += 1
                    osb = flpool.tile([P, 64], F32, tag="osb")
                    nc.vector.tensor_scalar(
                        out=osb[:],
                        in0=ps2[:],
                        scalar1=ii2_sb[:, t : t + 1],
                        scalar2=None,
                        op0=mybir.AluOpType.mult,
                    )
                    nc.sync.dma_start(out.ap()[t * P : (t + 1) * P, :], osb[:])

    nc.compile()
    return nc


# ---------------------------------------------------------------- entry point

_CACHE = {}


def kernel(x, edge_index, W1, W2):
    global LAST_RESULTS
    x = np.asarray(x)
    edge_index = np.asarray(edge_index)
    W1 = np.asarray(W1)
    W2 = np.asarray(W2)

    in_maps, meta = prep_inputs(x, edge_index, W1, W2, N_CORES)

    key = (meta["npad"], meta["B"], meta["G"])
    nc = _CACHE.get(key)
    if nc is None:
        nc = build_nc(meta, debug=False, enable_asserts=False)
        _CACHE[key] = nc

    res = bass_utils.run_bass_kernel_spmd(
        nc,
        in_maps,
        core_ids=list(range(meta["ncores"])),
        trace=TRACE,
    )
    LAST_RESULTS = res

    blocks = [res.results[c]["out"] for c in range(meta["ncores"])]
    full = np.concatenate(blocks, axis=0)
    return np.ascontiguousarray(full[: meta["n"], : meta["cout"]]).astype(np.float32)
